# revision 86
# baseline (speedup 1.0000x reference)
"""Trainium2 Bass kernel for multi-head causal attention + output projection.

Problem (hardcoded): B=4, T=2048, E=1024, H=16, D=64, float32.
  q = einsum('bte,hed->bhtd', data, Wq)   (same k, v)
  scores = q@k.T / sqrt(D), causal mask, softmax
  out = (attn @ v) concat-heads @ Wp + bp

Sharding across 8 NeuronCores: core c -> (batch b=c//2, head-group g=c%2).
Each core computes 8 heads of one batch and a partial projection with its
512-row slice of Wp; host sums the two partials per batch and adds bias.

Kernel structure (v2 — transposed attn@V):
  - all matmul operands bf16 (validated ~5e-3 rel err end to end)
  - scores kept transposed: psS[key, query] per key tile, 2 heads side by
    side; diagonal key tiles only compute query cols >= o*128, with the
    -1e30 causal mask folded into the PSUM accumulation as an extra
    ident.T@ntri matmul (no DVE masking, one PE->ACT->PE hop per tile)
  - attn@V with queries on PSUM partitions: stationary = exp weights
    [k, q-chunk], moving = V-augmented [k, 64+1] -> out [q, 65] per chunk
    (65-col moving beats the 512-col orientation ~2x in PE time); the ones
    column of V yields sum(exp) at col 64
  - PSUM zero-region rule: start=True lazily zeroes the whole 2KB bank, so
    each (bank, block) accumulation round has exactly one start (first
    matmul) and one stop (last matmul); PSUM is only read after the stop
  - normalization at block end: DVE reciprocal + per-partition
    tensor_scalar eviction, then a PE transpose ([q,c]->[c,q]) rebuilds
    olt[c, t] for the output projection
  - the PE is the overall bottleneck (~213us busy): remaining v/q/k
    projections, transposes and the output projection are drip-fed
    between key tiles by an EDF-rationed emission pacer so filler work
    interleaves with the ACT-paced attention cadence instead of bursting

PSUM discipline (8 banks):
  tag "pss"  x2 [128,1024]: score tiles (ping-pong) + head-phase groups
  tag "av"   x1 [128,1024]: per-block attn@V (A: cols 0:260, B: 512:772)
  tag "misc" x2 [128,512]:  fillers (v pass B, q/k blocks, transposes,
                            projection rounds), ping-pong hides WAR
"""

from collections import deque

import numpy as np

import concourse.bass as bass
import concourse.mybir as mybir
import concourse.tile as tile
from contextlib import ExitStack

F32 = mybir.dt.float32
BF16 = mybir.dt.bfloat16

# Full-problem constants
B, T, E, H, D = 4, 2048, 1024, 16, 64
N_CORES = 8
H_LOC = H // 2          # heads per core
HP = H_LOC // 2         # head pairs per core
SCALE = float(D) ** -0.5

C = H_LOC * D           # local concat width (512)
ET = E // 128           # embedding 128-tiles (8)
TT = T // 128           # token 128-tiles (16)
TQB = 512               # query-block width
NJB = T // TQB          # query blocks (4)
VW = 65                 # vaug per-head width (64 + ones col)
LAG = 4                 # tiles between exp and attn@V consumption (the
                        # loop emits AV one tile later -> effective 5)

# pacing constants (ns estimates mirroring the cost model)
PE_C = 1.0 / 2.4
ACT_C = 1.0 / 1.2


def _exp_ns(cols, nops=1):
    return cols * ACT_C + 185.0 * nops


def build_program(nc):
    AF = mybir.ActivationFunctionType
    AL = mybir.AluOpType

    xTd = nc.dram_tensor("xT", [E, T], BF16, kind="ExternalInput").ap()
    wqd = nc.dram_tensor("wq", [E, C], BF16, kind="ExternalInput").ap()
    wkd = nc.dram_tensor("wk", [E, C], BF16, kind="ExternalInput").ap()
    wvd = nc.dram_tensor("wv", [E, C], BF16, kind="ExternalInput").ap()
    wpd = nc.dram_tensor("wp", [C, E], BF16, kind="ExternalInput").ap()
    # [128,256]: two copies of tri[r, g] = (g >= r)
    maskd = nc.dram_tensor("masks", [128, 256], BF16, kind="ExternalInput").ap()
    identd = nc.dram_tensor("ident", [128, 128], BF16, kind="ExternalInput").ap()
    out = nc.dram_tensor("out", [T, E], F32, kind="ExternalOutput").ap()

    with tile.TileContext(nc) as tc, ExitStack() as ctx:
        const = ctx.enter_context(tc.tile_pool(name="const", bufs=1))
        xt_pool = ctx.enter_context(tc.tile_pool(name="xt", bufs=ET))
        wv_pool = ctx.enter_context(tc.tile_pool(name="wvp", bufs=ET))
        wq_pool = ctx.enter_context(tc.tile_pool(name="wqp", bufs=ET))
        wk_pool = ctx.enter_context(tc.tile_pool(name="wkp", bufs=ET))
        vaug_pool = ctx.enter_context(tc.tile_pool(name="vaugp", bufs=1))
        qk_pool = ctx.enter_context(tc.tile_pool(name="qkp", bufs=2 * HP))
        ee_pool = ctx.enter_context(tc.tile_pool(name="eep", bufs=8))
        usb_pool = ctx.enter_context(tc.tile_pool(name="usbp", bufs=8))
        r_pool = ctx.enter_context(tc.tile_pool(name="rp", bufs=4))
        olt_pool = ctx.enter_context(tc.tile_pool(name="oltp", bufs=HP))
        wp_pool = ctx.enter_context(tc.tile_pool(name="wpp", bufs=HP))
        out_pool = ctx.enter_context(tc.tile_pool(name="outp", bufs=4))
        psum = ctx.enter_context(tc.tile_pool(name="ps", bufs=4, space="PSUM"))

        mask_sb = const.tile([128, 256], BF16, name="mask_sb")
        ident_sb = const.tile([128, 128], BF16, name="ident_sb")

        vaug = vaug_pool.tile([128, TT * H_LOC * VW], BF16, name="vaug")
        xt = [xt_pool.tile([128, T], BF16, tag="xt", name=f"xt{e}")
              for e in range(ET)]
        wvt = [wv_pool.tile([128, C], BF16, tag="wv", name=f"wvt{e}")
               for e in range(ET)]
        wqt = [wq_pool.tile([128, C], BF16, tag="wq", name=f"wqt{e}")
               for e in range(ET)]
        wkt = [wk_pool.tile([128, C], BF16, tag="wk", name=f"wkt{e}")
               for e in range(ET)]
        qT = [qk_pool.tile([128, T], BF16, tag="qk", name=f"qT{p}")
              for p in range(HP)]
        kT = [qk_pool.tile([128, T], BF16, tag="qk", name=f"kT{p}")
              for p in range(HP)]
        olt = [olt_pool.tile([128, T], BF16, tag="olt", name=f"olt{c}")
               for c in range(HP)]
        wpt = [wp_pool.tile([128, E], BF16, tag="wp", name=f"wpt{c}")
               for c in range(HP)]

        TAG_BUFS = {"pss": 2, "av": 1, "misc": 2}

        def ps_tile(tag, name):
            # pss/av slots are 2 banks ([128,1024] f32); misc slots 1 bank
            shape = [128, 512] if tag == "misc" else [128, 1024]
            return psum.tile(shape, F32, tag=tag, name=name,
                             bufs=TAG_BUFS[tag])

        # ---------------- input DMAs (order = consumption order) ----------
        # first v matmul needs only xt0[:,0:128]+wvt0: tiny first transfers
        nc.sync.dma_start(xt[0][:, 0:128], xTd[0:128, 0:128])
        nc.sync.dma_start(wvt[0][:], wvd[0:128, :])
        nc.sync.dma_start(xt[0][:, 128:T // 2], xTd[0:128, 128:T // 2])
        for e in range(1, ET):
            nc.sync.dma_start(wvt[e][:], wvd[e * 128:(e + 1) * 128, :])
            nc.sync.dma_start(xt[e][:, 0:T // 2],
                              xTd[e * 128:(e + 1) * 128, 0:T // 2])
        for e in range(ET):
            nc.sync.dma_start(wqt[e][:], wqd[e * 128:(e + 1) * 128, :])
        for e in range(ET):
            nc.sync.dma_start(xt[e][:, T // 2:T],
                              xTd[e * 128:(e + 1) * 128, T // 2:T])
        for e in range(ET):
            nc.sync.dma_start(wkt[e][:], wkd[e * 128:(e + 1) * 128, :])
        nc.sync.dma_start(mask_sb[:], maskd)
        nc.sync.dma_start(ident_sb[:], identd)
        for c in range(HP):
            nc.sync.dma_start(wpt[c][:], wpd[c * 128:(c + 1) * 128, :])

        # ones columns of vaug (data cols are fully overwritten by evicts)
        nc.vector.memset(vaug[:, 64:TT * H_LOC * VW:VW], 1.0)

        def evict_engine(i, with_act=False):
            # GPSIMD cannot access PSUM on this target: evictions are
            # DVE-only during attention, DVE/ACT alternating in phases
            # where the ACT (exp) is idle.
            engs = [nc.vector, nc.scalar] if with_act else [nc.vector]
            eng = engs[i % len(engs)]

            def copy(out_ap, in_ap, _eng=eng):
                if _eng is nc.scalar:
                    return _eng.copy(out_ap, in_ap)
                return _eng.tensor_copy(out_ap, in_ap)

            def ts(out_ap, in_ap, scalar, _unused, _op, _eng=eng):
                if _eng is nc.scalar:
                    return _eng.mul(out_ap, in_ap, scalar)
                return _eng.tensor_scalar(out_ap, in_ap, scalar, None, _op)
            return type("E", (), {"tensor_copy": staticmethod(copy),
                                  "tensor_scalar": staticmethod(ts)})

        # ---------------- v projection ------------------------------------
        def v_mm(pv, slot, e, t):
            nc.tensor.matmul(
                pv[:, slot * C:(slot + 1) * C],
                xt[e][:, t * 128:(t + 1) * 128],
                wvt[e][:],
                start=(e == 0), stop=(e == ET - 1))

        def v_evict(pv, slot, t, eng):
            base = t * H_LOC * VW
            dst = vaug[:, base:base + H_LOC * VW].rearrange(
                "p (h c) -> p h c", c=VW)[:, :, 0:64]
            src = pv[:, slot * C:(slot + 1) * C].rearrange(
                "p (h c) -> p h c", c=64)
            eng.tensor_copy(dst, src)

        # head phase: t 0..5 in 4 interleaved groups, then t 6..7
        def emit_v_pass_a():
            g0 = ps_tile("pss", "psv_a0")
            g1 = ps_tile("pss", "psv_a1")
            g2 = ps_tile("misc", "psv_a2")
            g3 = ps_tile("misc", "psv_a3")
            gs = [(g0, 0), (g0, 1), (g1, 0), (g1, 1), (g2, 0), (g3, 0)]
            for e in range(ET):
                for t in range(6):
                    v_mm(gs[t][0], gs[t][1], e, t)
            for t in range(6):
                v_evict(gs[t][0], gs[t][1], t, evict_engine(t, with_act=True))
            g4 = ps_tile("av", "psv_a4")
            for e in range(ET):
                for t in (6, 7):
                    v_mm(g4, t % 2, e, t)
            for t in (6, 7):
                v_evict(g4, t % 2, t, evict_engine(t, with_act=True))

        # ---------------- q/k projections (head phase, pss tag) -----------
        def emit_qk_block(wlist, dst, p, jbp, eng, tag="pss"):
            pq = ps_tile(tag, "psqk")
            for e in range(ET):
                for j in range(2):
                    jb = jbp + j
                    nc.tensor.matmul(
                        pq[:, j * TQB:(j + 1) * TQB],
                        wlist[e][:, p * 128:(p + 1) * 128],
                        xt[e][:, jb * TQB:(jb + 1) * TQB],
                        start=(e == 0), stop=(e == ET - 1))
            eng.tensor_copy(dst[p][:, jbp * TQB:(jbp + 2) * TQB], pq[:])

        # ---------------- pacer / filler machinery ------------------------
        # Two queues: `urgent` (per-chunk transposes — tiny, gate olt) and
        # `background` (v pass B, q/k projections, output projection —
        # clock-paced against the ACT (exp) bottleneck). Entries:
        # (key, gen, min_tick): min_tick delays emission until the DVE work
        # they depend on has had time to execute (avtick = AV emissions).
        clock = {"pe": 0.0, "act": 0.0}
        avtick = [0]
        urgent = deque()
        background = deque()

        def gen_v_group_b(t):
            pv = ps_tile("misc", f"psv_b{t}")
            for e in range(ET):
                v_mm(pv, 0, e, t)
                yield TQB * PE_C
            v_evict(pv, 0, t, evict_engine(t))

        def gen_qk_fill(wlist, dst, p, jb):
            pq = ps_tile("misc", "psqkf")
            for e in range(ET):
                nc.tensor.matmul(
                    pq[:],
                    wlist[e][:, p * 128:(p + 1) * 128],
                    xt[e][:, jb * TQB:(jb + 1) * TQB],
                    start=(e == 0), stop=(e == ET - 1))
                yield TQB * PE_C
            evict_engine(p + jb).tensor_copy(
                dst[p][:, jb * TQB:(jb + 1) * TQB], pq[:])

        def gen_transp_chunk(p, jb, c, usb_c):
            mt = psum.tile([128, 128], BF16, tag="misc", name="pstr",
                           bufs=TAG_BUFS["misc"])
            nc.tensor.transpose(mt[:], usb_c[:], ident_sb[:])
            evict_engine(c).tensor_copy(
                olt[p][:, jb * TQB + c * 128:jb * TQB + (c + 1) * 128],
                mt[:])
            yield 128 * PE_C

        def gen_proj_tile(t, act_evict=False):
            ot = out_pool.tile([128, E], F32, tag="out", name=f"ot{t}")
            for nb in range(2):
                mp = ps_tile("misc", "psproj")
                h = nb * TQB
                for cc in range(HP):
                    nc.tensor.matmul(
                        mp[:],
                        olt[cc][:, t * 128:(t + 1) * 128],
                        wpt[cc][:, h:h + TQB],
                        start=(cc == 0), stop=(cc == HP - 1))
                    yield TQB * PE_C
                # ACT is idle at the very end: offload the evictions
                if act_evict:
                    nc.scalar.copy(ot[:, h:h + TQB], mp[:])
                else:
                    nc.vector.tensor_copy(ot[:, h:h + TQB], mp[:])
                nc.sync.dma_start(out[t * 128:(t + 1) * 128, h:h + TQB],
                                  ot[:, h:h + TQB])

        def step_q(q):
            entry = q[0]
            try:
                clock["pe"] += next(entry[1])
                if len(entry) == 5:
                    entry[4][0] -= 1
                return True
            except StopIteration:
                q.popleft()
                return False

        quota = [0.0]

        def pace():
            while urgent and urgent[0][2] <= avtick[0]:
                step_q(urgent)
            if not background:
                return
            # EDF rationing: per tick emit just enough background steps that
            # every entry finishes by its deadline, spread uniformly
            tick = avtick[0]
            cum, rate = 0.0, 0.0
            for entry in background:
                cum += entry[4][0]
                rate = max(rate, cum / max(entry[3] - tick, 1.0))
            quota[0] = min(quota[0] + max(rate, 1.5), 8.0)
            while (background and quota[0] >= 1.0
                   and background[0][2] <= avtick[0]):
                if step_q(background):
                    quota[0] -= 1.0

        def force_drain(q, pred):
            """Fully emit all entries of q matching pred (FIFO order, so
            everything queued before them drains too)."""
            while any(pred(e[0]) for e in q):
                step_q(q)

        def drain_fillers():
            # interleave: one full transpose unit, then one projection tile
            # (9 steps), so the projection matmuls hide the next
            # transpose's DVE wait while keeping transp-before-its-proj
            while urgent or background:
                if urgent:
                    while step_q(urgent):
                        pass  # complete exactly one unit
                for _ in range(9):
                    if not background:
                        break
                    step_q(background)

        # ---------------- attention block ---------------------------------
        def emit_attn_block(p, jb, bi):
            n_tk = NJB * (jb + 1)
            av = ps_tile("av", "psav")
            r_t = [r_pool.tile([128, NJB], F32, tag="r", name=f"r{h}")
                   for h in range(2)]
            usb = [None] * NJB
            pend = deque()

            def emit_av():
                avtick[0] += 1
                t, ee = pend.popleft()
                o = t - NJB * jb
                for h in range(2):
                    hb = h * 512
                    vcol = t * H_LOC * VW + (2 * p + h) * VW
                    for cch in range(max(o, 0), NJB):
                        # one accumulation group per PSUM bank per block:
                        # start only on the round's first matmul (start
                        # marks the whole 2KB zero-region pending; later
                        # chunks are lazily zeroed on first write), stop
                        # only on the last (tile n_tk-1 touches chunk 3
                        # alone)
                        nc.tensor.matmul(
                            av[:, hb + cch * VW:hb + cch * VW + VW],
                            ee[:, hb + cch * 128:hb + (cch + 1) * 128],
                            vaug[:, vcol:vcol + VW],
                            start=(t == 0 and cch == max(o, 0)),
                            stop=(t == n_tk - 1 and cch == NJB - 1))
                        clock["pe"] += VW * PE_C
                # after the bank groups stop (last tile): normalize + evict
                # all chunks, then queue their transposes / projections
                if t == n_tk - 1:
                    for h in range(2):
                        hb = h * 512
                        nc.vector.reciprocal(
                            r_t[h][:],
                            av[:, hb + 64:hb + NJB * VW:VW])
                    for cch in range(NJB):
                        usb[cch] = usb_pool.tile(
                            [128, 128], BF16, tag="usb", name=f"usb{cch}")
                        for h in range(2):
                            hb = h * 512
                            eng = evict_engine(cch + h,
                                               with_act=(p == HP - 1))
                            eng.tensor_scalar(
                                usb[cch][:, h * 64:(h + 1) * 64],
                                av[:, hb + cch * VW:hb + cch * VW + 64],
                                r_t[h][:, cch:cch + 1], None, AL.mult)
                        urgent.append((("t", bi),
                                       gen_transp_chunk(p, jb, cch, usb[cch]),
                                       avtick[0] + 3 + cch))
                        if p == HP - 1:
                            # drain each block's proj within the following
                            # pair-3 block (last: by end)
                            dl = {0: 132, 1: 144, 2: 160, 3: 160}[jb]
                            background.append(
                                (("proj", jb),
                                 gen_proj_tile(NJB * jb + cch, jb == NJB - 1),
                                 avtick[0] + 4 + cch, dl, [9]))

            for t in range(n_tk):
                if len(pend) > LAG:
                    emit_av()
                o = t - NJB * jb
                psS = ps_tile("pss", "psS")
                lo = max(o, 0) * 128
                for h in range(2):
                    hb = h * 512
                    kc = kT[p][h * 64:(h + 1) * 64, t * 128:(t + 1) * 128]
                    if o < 0:
                        nc.tensor.matmul(
                            psS[:, hb:hb + 512], kc,
                            qT[p][h * 64:(h + 1) * 64,
                                  jb * TQB:(jb + 1) * TQB],
                            start=True, stop=True)
                    else:
                        # diagonal tile: restricted query columns only;
                        # the within-tile triangle is masked on DVE after
                        # the exp (LAG absorbs the extra hop)
                        nc.tensor.matmul(
                            psS[:, hb + lo:hb + 512], kc,
                            qT[p][h * 64:(h + 1) * 64,
                                  jb * TQB + lo:(jb + 1) * TQB],
                            start=True, stop=True)
                    clock["pe"] += (512 - lo) * PE_C
                ee = ee_pool.tile([128, 1024], BF16, tag="ee", name="ee")
                if o <= 0:
                    nc.scalar.activation(ee[:], psS[:], AF.Exp, scale=SCALE)
                    clock["act"] += _exp_ns(1024)
                else:
                    for h in range(2):
                        nc.scalar.activation(
                            ee[:, h * 512 + lo:h * 512 + 512],
                            psS[:, h * 512 + lo:h * 512 + 512],
                            AF.Exp, scale=SCALE)
                    clock["act"] += _exp_ns(2 * (512 - lo), nops=2)
                if o >= 0:
                    band = ee[:].rearrange("p (h q) -> p h q", q=512)[
                        :, :, o * 128:(o + 1) * 128]
                    mband = mask_sb[:].rearrange("p (h q) -> p h q", q=128)
                    nc.vector.tensor_tensor(band, band, mband, AL.mult)
                pend.append((t, ee))
                pace()
            while pend:
                emit_av()

        # ---------------- emission ----------------------------------------
        emit_v_pass_a()
        emit_qk_block(wqt, qT, 0, 0, evict_engine(0, True))
        emit_qk_block(wqt, qT, 0, 2, evict_engine(1, True))
        emit_qk_block(wkt, kT, 0, 0, evict_engine(0, True))
        emit_qk_block(wkt, kT, 0, 2, evict_engine(1, True))

        for tg in range(8, TT):
            jbn = tg // 4
            dl = max(2 * jbn * (jbn + 1) - 2, 1)
            background.append((("vb", tg), gen_v_group_b(tg), 0, dl, [9]))
        for p in range(1, HP):
            for jb in range(NJB):
                s_blk = 40 * p + 2 * jb * (jb + 1)
                background.append(
                    (("qk", p, jb), gen_qk_fill(wqt, qT, p, jb), 0,
                     max(s_blk - 2, 1), [9]))
                background.append(
                    (("qk", p, jb), gen_qk_fill(wkt, kT, p, jb), 0,
                     max(s_blk + 4 * jb - 2, 1), [9]))
        assert [e[0] for e in background if e[0][0] == "qk"] == [
            ("qk", p, jb) for p in range(1, HP) for jb in range(NJB)
            for _ in range(2)]

        bi = 0
        for p in range(HP):
            for jb in range(NJB):
                # correctness: everything this block consumes must already
                # be emitted (Tile deps follow emission order) — vaug tiles
                # for its key range, q/k of this pair; plus recycle old
                # transposes (usb pool depth) before new norms allocate.
                n_tk = NJB * (jb + 1)
                force_drain(background, lambda k, n=n_tk, p=p, jb=jb: (
                    (k[0] == "vb" and k[1] < n)
                    or (k[0] == "qk" and (k[1] < p
                                          or (k[1] == p and k[2] <= jb)))))
                force_drain(urgent, lambda k, bi=bi: (
                    k[0] == "t" and k[1] <= bi - 2))
                emit_attn_block(p, jb, bi)
                bi += 1
        drain_fillers()
    return nc


def make_host_inputs():
    import ml_dtypes
    tri = np.where(np.arange(128)[None, :] >= np.arange(128)[:, None],
                   1.0, 0.0).astype(np.float32)
    masks = np.concatenate([tri, tri], axis=1).astype(ml_dtypes.bfloat16)
    ident = np.eye(128, dtype=np.float32).astype(ml_dtypes.bfloat16)
    return masks, ident


def shard_inputs(data, Wq, Wk, Wv, Wp):
    """Build the 8 per-core input maps from full inputs."""
    import ml_dtypes
    BF = ml_dtypes.bfloat16
    data = np.asarray(data, np.float32)
    Wq = np.asarray(Wq, np.float32)
    Wk = np.asarray(Wk, np.float32)
    Wv = np.asarray(Wv, np.float32)
    Wp = np.asarray(Wp, np.float32)
    masks, ident = make_host_inputs()
    in_maps = []
    for c in range(N_CORES):
        b, g = c // 2, c % 2
        hs = slice(g * H_LOC, (g + 1) * H_LOC)
        in_maps.append({
            "xT": np.ascontiguousarray(data[b].T).astype(BF),
            "wq": np.ascontiguousarray(
                Wq[hs].transpose(1, 0, 2).reshape(E, H_LOC * D)).astype(BF),
            "wk": np.ascontiguousarray(
                Wk[hs].transpose(1, 0, 2).reshape(E, H_LOC * D)).astype(BF),
            "wv": np.ascontiguousarray(
                Wv[hs].transpose(1, 0, 2).reshape(E, H_LOC * D)).astype(BF),
            "wp": np.ascontiguousarray(
                Wp[g * H_LOC * D:(g + 1) * H_LOC * D, :]).astype(BF),
            "masks": masks,
            "ident": ident,
        })
    return in_maps


_NC_CACHE = {}


def legalize_single_wait(nc):
    """This toolchain's walrus accepts at most ONE sync wait per engine
    instruction; Tile freely emits more. Split extra waits onto preceding
    same-engine NoOps (engine FIFOs make that equivalent)."""
    import bass_rust
    cnt = 0
    for f in nc.m.functions:
        for blk in f.blocks:
            new = []
            changed = False
            for inst in blk.instructions:
                si = inst.sync_info
                if si is not None and len(si.on_wait) > 1:
                    waits = list(si.on_wait)
                    for w in waits[:-1]:
                        nop = bass_rust.InstNoOp(name=f"legal_nop_{cnt}")
                        cnt += 1
                        nop.engine = inst.engine
                        nop.sync_info = bass_rust.SyncInfo(on_wait=[w],
                                                           on_update=[])
                        new.append(nop)
                    inst.sync_info = bass_rust.SyncInfo(
                        on_wait=[waits[-1]], on_update=list(si.on_update))
                    changed = True
                new.append(inst)
            if changed:
                blk.instructions = new
    return cnt


def get_nc():
    if "nc" not in _NC_CACHE:
        nc = bass.Bass("TRN2", target_bir_lowering=False, debug=False,
                       num_devices=N_CORES)
        build_program(nc)
        legalize_single_wait(nc)
        _NC_CACHE["nc"] = nc
    return _NC_CACHE["nc"]


def run(inputs, trace=False, **kw):
    """Run on the 8 NeuronCores; returns (full_output, BassKernelResults)."""
    from concourse.bass_utils import run_bass_kernel_spmd
    nc = get_nc()
    in_maps = shard_inputs(inputs["data"], inputs["Wq"], inputs["Wk"],
                           inputs["Wv"], inputs["Wp"])
    res = run_bass_kernel_spmd(nc, in_maps, core_ids=list(range(N_CORES)),
                               trace=trace, **kw)
    bp = np.asarray(inputs["bp"], np.float32)
    outf = np.empty((B, T, E), np.float32)
    for b in range(B):
        outf[b] = res.results[2 * b]["out"] + res.results[2 * b + 1]["out"] + bp
    return outf, res


def kernel(**inputs):
    out, _ = run(inputs)
    return out
